# revision 1
# baseline (speedup 1.0000x reference)
"""Trainium2 Bass kernel for nn_CustomConvLayer (bilinear-tap conv).

Math: each of the K=9 taps gathers x at constant sub-pixel offset
(dy, dx) via separable bilinear interpolation, scales by a per-
(cout, cin, tap) weight, and accumulates over taps and input channels.

Fast path (all taps share the same fractional offset, as in the
reference's base+0.4 grid): the op factors exactly into
    out = conv_int(blend_x(blend_y(xp)), W3)
where blend_y/x are the shared 2-tap bilinear blends applied once to
the padded input and W3[o,i,iy,ix] = sum_k w[o,i,k] at the integer tap
positions (x the blend normalization). This cuts TensorE work from 12
banded 128x128 blocks (4x4 folded filter) to 6 (3x3 filter); the
blends run as two fused scalar_tensor_tensor passes on DVE.

Distribution: data-parallel over batch, one image per NeuronCore.
Per core the conv runs as TensorE matmuls:
  - SBUF layout: even padded rows on partitions 0-63 (64 channels),
    odd rows on partitions 64-127; 65 row-pair segments x 132 cols;
    contraction K = 128 = (2 rows x 64 cin).
  - The input is staged twice (row r and row r+1 layouts) so the
    y-blend is lane-local; both copies are host-packed into the exact
    SBUF image (bf16, zeros baked in) so each load is one big
    contiguous DMA and no device memsets are needed.
  - M = 128 = (2 output rows x 64 cout), N = 512 = (4 output row-pairs
    x W=128) per matmul, PSUM-accumulated over the 6 (delta, v) blocks.
  - PSUM drains to bf16 stage tiles on the Scalar engine; the output
    leaves in stage layout and is unpermuted to [Cout,H,W] f32 on host.

General fallback (arbitrary per-tap offsets): dense folded conv of
footprint up to 5x5, f32r, as in the original baseline.
"""

import numpy as np
import ml_dtypes

import concourse.bass as bass  # noqa: F401
import concourse.mybir as mybir
import concourse.tile as tile
from concourse import bacc
from concourse.bass_utils import run_bass_kernel_spmd

B, CIN, H, W = 8, 64, 128, 128
COUT, KTAPS = 64, 9
NCORES = 8

# fast-path SBUF geometry
WP = 132          # padded segment width (xp cols 0..131; cols >=129 zero)
NSEG = 65         # row-pair segments (xp rows 2t+j, t in 0..64)
CHUNKS = ((0, 22), (22, 44), (44, 65))

BF16 = ml_dtypes.bfloat16


# --------------------------------------------------------------------
# planning (host)
# --------------------------------------------------------------------

def fold_weights(weights, tap_offsets):
    """General path: fold per-tap scalar weights + bilinear coeffs into
    W_eff [COUT, CIN, Fy, Fx] (float64)."""
    w = np.asarray(weights, np.float64)
    off = np.asarray(tap_offsets, np.float64)
    dy, dx = off[:, 0], off[:, 1]
    assert (dy >= 0).all() and (dx >= 0).all(), "negative tap offsets unsupported"
    iy = np.floor(dy).astype(np.int64)
    fy = dy - iy
    ix = np.floor(dx).astype(np.int64)
    fx = dx - ix
    Fy = int(iy.max()) + 2
    Fx = int(ix.max()) + 2
    assert Fy <= 5 and Fx <= 5
    Weff = np.zeros((COUT, CIN, Fy, Fx))
    for k in range(KTAPS):
        for a, cy in ((0, 1.0 - fy[k]), (1, fy[k])):
            for bb, cx in ((0, 1.0 - fx[k]), (1, fx[k])):
                Weff[:, :, iy[k] + a, ix[k] + bb] += w[:, :, k] * (cy * cx)
    return Weff


def make_blocks(Weff):
    """Build the (delta, v) lhsT blocks of the banded row-pair matmul
    structure.

    Block (delta, v) couples input row-pair t = 4*sg + g + delta to
    output row-pair 4*sg + g:  lhsT[(j, ci), (i, co)] = W_eff[co, ci,
    u = 2*delta + j - i, v] (zero when u out of range).

    Returns (blocks, Wh): blocks is a list of (delta, v); Wh is
    [128, nblk*128] float64 with block bi at columns bi*128:(bi+1)*128.
    """
    _, _, Fy, Fx = Weff.shape
    ndelta = Fy // 2 + 1
    blocks, mats = [], []
    for d in range(ndelta):
        for v in range(Fx):
            Mb = np.zeros((128, 128))
            nz = False
            for j in (0, 1):
                for i in (0, 1):
                    u = 2 * d + j - i
                    if 0 <= u < Fy:
                        blk = Weff[:, :, u, v].T  # [cin, cout]
                        Mb[j * 64:(j + 1) * 64, i * 64:(i + 1) * 64] = blk
                        nz = nz or bool(np.abs(blk).max() > 0)
            if nz:
                blocks.append((d, v))
                mats.append(Mb)
    Wh = np.stack(mats, 0).transpose(1, 0, 2).reshape(128, -1)
    return blocks, np.ascontiguousarray(Wh)


def plan_from_inputs(weights, tap_offsets):
    """Decide fast vs general path and precompute the weight blocks.

    Returns a dict with everything build() and make_in_maps() need.
    The dict's "key" entry is hashable and identifies the compiled
    module (block structure + blend scalars' structure)."""
    w = np.asarray(weights, np.float64)
    off = np.asarray(tap_offsets, np.float64)
    dy, dx = off[:, 0], off[:, 1]
    iy = np.floor(dy).astype(np.int64)
    fy = dy - iy
    ix = np.floor(dx).astype(np.int64)
    fx = dx - ix
    fast = (
        (dy >= 0).all() and (dx >= 0).all()
        and float(np.ptp(fy)) < 1e-5 and float(np.ptp(fx)) < 1e-5
        and int(iy.max()) <= 2 and int(ix.max()) <= 2
    )
    if not fast:
        Weff = fold_weights(weights, tap_offsets)
        blocks, Wh = make_blocks(Weff)
        return {
            "mode": "general",
            "blocks": blocks,
            "Wh": Wh,
            "key": ("general", tuple(blocks)),
        }

    fy0, fx0 = float(fy.mean()), float(fx.mean())
    # role flags: blend = base + s * shifted, with the larger bilinear
    # coefficient on the unscaled operand (keeps s <= 1)
    y_swap = fy0 > 0.5          # unscaled operand is row r+1
    x_swap = fx0 > 0.5          # unscaled operand is col c+1
    sy = fy0 / (1.0 - fy0) if not y_swap else (1.0 - fy0) / fy0
    sx = fx0 / (1.0 - fx0) if not x_swap else (1.0 - fx0) / fx0
    scale = ((1.0 - fy0) if not y_swap else fy0) * (
        (1.0 - fx0) if not x_swap else fx0)

    Fy = int(iy.max()) + 1
    Fx = int(ix.max()) + 1
    W3 = np.zeros((COUT, CIN, Fy, Fx))
    for k in range(KTAPS):
        W3[:, :, iy[k], ix[k]] += w[:, :, k]
    W3 *= scale
    blocks, Wh = make_blocks(W3)
    return {
        "mode": "fast",
        "blocks": blocks,
        "Wh": Wh,
        "sy": sy,
        "sx": sx,
        "y_swap": y_swap,
        "x_swap": x_swap,
        "key": ("fast", tuple(blocks), round(sy, 9), round(sx, 9),
                y_swap, x_swap),
    }


# --------------------------------------------------------------------
# device program
# --------------------------------------------------------------------

def build_fast(plan, repeat=1):
    """3x3 integer conv (6 banded blocks, bf16) + on-chip bilinear
    blend via two fused scalar_tensor_tensor DVE passes."""
    blocks = plan["blocks"]
    nblk = len(blocks)
    sy, sx = plan["sy"], plan["sx"]
    dt = mybir.dt.bfloat16
    nc = bacc.Bacc(
        "TRN2", target_bir_lowering=False, debug=False, enable_asserts=False
    )
    xa = nc.dram_tensor("xa", [128, NSEG * WP], dt, kind="ExternalInput")
    xb2 = nc.dram_tensor("xb2", [128, NSEG * WP], dt, kind="ExternalInput")
    wb = nc.dram_tensor("wb", [128, nblk * 128], dt, kind="ExternalInput")
    out = nc.dram_tensor("out", [128, 16 * 512], dt, kind="ExternalOutput")

    with tile.TileContext(nc) as tc:
        with (
            tc.tile_pool(name="const", bufs=1) as const_pool,
            tc.tile_pool(name="big", bufs=2) as big_pool,
            tc.tile_pool(name="psum", bufs=8, space="PSUM") as psum_pool,
            tc.tile_pool(name="stage", bufs=4) as stage_pool,
        ):
            wbuf = const_pool.tile([128, nblk * 128], dt, tag="wbuf")
            xav = xa.ap().rearrange("p (t c) -> p t c", c=WP)
            xbv = xb2.ap().rearrange("p (t c) -> p t c", c=WP)
            outv = out.ap().rearrange("p (s n) -> p s n", n=512)

            for _rep in range(repeat):
                abuf = big_pool.tile([128, NSEG, WP], dt, tag="abuf",
                                     name=f"abuf_{_rep}")
                bbuf = big_pool.tile([128, NSEG, WP], dt, tag="bbuf",
                                     name=f"bbuf_{_rep}")
                rybuf = big_pool.tile([128, NSEG, WP], dt, tag="rybuf",
                                      name=f"rybuf_{_rep}")
                xbuf = big_pool.tile([128, NSEG, WP], dt, tag="xbuf",
                                     name=f"xbuf_{_rep}")

                nc.sync.dma_start(out=wbuf[:], in_=wb.ap())
                for t0, t1 in CHUNKS:
                    nc.sync.dma_start(out=abuf[:, t0:t1], in_=xav[:, t0:t1])
                    nc.sync.dma_start(out=bbuf[:, t0:t1], in_=xbv[:, t0:t1])

                # blend: ry = base + sy * shifted (y), then
                #        xb = base + sx * shifted (x). The fused
                #        scalar_tensor_tensor has NO DVE fast modes
                #        (1 elem/cy), so split each blend into
                #        tensor_scalar (4x in bf16-SBUF) + tensor_tensor
                #        (2x): 13.2us -> well under the PE envelope.
                scr = big_pool.tile([128, NSEG, WP], dt, tag="scr",
                                    name=f"scr_{_rep}")
                for t0, t1 in CHUNKS:
                    ya, yb = abuf[:, t0:t1], bbuf[:, t0:t1]
                    y_sc, y_base = (ya, yb) if plan["y_swap"] else (yb, ya)
                    nc.vector.tensor_scalar(
                        scr[:, t0:t1], y_sc, sy, None,
                        op0=mybir.AluOpType.mult,
                    )
                    nc.vector.tensor_tensor(
                        rybuf[:, t0:t1], scr[:, t0:t1], y_base,
                        op=mybir.AluOpType.add,
                    )
                    r0 = rybuf[:, t0:t1, 0:131]
                    r1 = rybuf[:, t0:t1, 1:132]
                    x_sc, x_base = (r0, r1) if plan["x_swap"] else (r1, r0)
                    nc.vector.tensor_scalar(
                        scr[:, t0:t1, 0:131], x_sc, sx, None,
                        op0=mybir.AluOpType.mult,
                    )
                    nc.vector.tensor_tensor(
                        xbuf[:, t0:t1, 0:131], scr[:, t0:t1, 0:131], x_base,
                        op=mybir.AluOpType.add,
                    )

                for half in range(2):
                    ptiles = [
                        psum_pool.tile(
                            [128, 4, 128], mybir.dt.float32, tag="ps",
                            name=f"ps_{_rep}_{half}_{k}",
                        )
                        for k in range(8)
                    ]
                    for bi, (d, v) in enumerate(blocks):
                        lhsT = wbuf[:, bi * 128:(bi + 1) * 128]
                        first = bi == 0
                        last = bi == nblk - 1
                        for sg in range(half * 8, half * 8 + 8):
                            t0 = 4 * sg + d
                            rhs = xbuf[:, t0:t0 + 4, v:v + 128]
                            nc.tensor.matmul(
                                ptiles[sg % 8][:], lhsT, rhs,
                                start=first, stop=last,
                            )
                    for q in range(2):
                        st = stage_pool.tile(
                            [128, 4, 512], dt, tag="st",
                            name=f"st_{_rep}_{half}_{q}",
                        )
                        for g in range(4):
                            sg = half * 8 + q * 4 + g
                            nc.scalar.copy(
                                out=st[:, g],
                                in_=ptiles[sg % 8][:].rearrange(
                                    "p a b -> p (a b)"),
                            )
                        nc.gpsimd.dma_start(
                            out=outv[:, (half * 2 + q) * 4:
                                     (half * 2 + q) * 4 + 4],
                            in_=st[:],
                        )

    nc.compile()
    return nc


def build_general(plan, repeat=1):
    """Original baseline: dense folded conv (up to 12 banded blocks),
    f32r, device-side padding memsets."""
    blocks = plan["blocks"]
    nblk = len(blocks)
    dt_x = mybir.dt.float32r
    ms_cast = lambda ap: ap.bitcast(mybir.dt.float32)  # noqa: E731
    GWP, GNSEG = 132, 66
    nc = bacc.Bacc(
        "TRN2", target_bir_lowering=False, debug=False, enable_asserts=False
    )
    xs = nc.dram_tensor("xs", [CIN, H, W], dt_x, kind="ExternalInput")
    wb = nc.dram_tensor("wb", [128, nblk * 128], dt_x, kind="ExternalInput")
    out = nc.dram_tensor("out", [COUT, H, W], mybir.dt.float32,
                         kind="ExternalOutput")

    with tile.TileContext(nc) as tc:
        with (
            tc.tile_pool(name="const", bufs=1) as const_pool,
            tc.tile_pool(name="psum", bufs=8, space="PSUM") as psum_pool,
            tc.tile_pool(name="stage", bufs=8) as stage_pool,
        ):
            xbuf = const_pool.tile([128, GNSEG * GWP], dt_x, tag="xbuf")
            wbuf = const_pool.tile([128, nblk * 128], dt_x, tag="wbuf")
            xv = xbuf[:].rearrange("p (t c) -> p t c", c=GWP)

            for _rep in range(repeat):
                nc.sync.dma_start(out=wbuf[:], in_=wb.ap())

                nc.gpsimd.memset(ms_cast(xv[:, :, 0:1]), 0.0)
                nc.gpsimd.memset(ms_cast(xv[:, :, 129:132]), 0.0)
                nc.gpsimd.memset(ms_cast(xv[0:64, 0:1, :]), 0.0)
                nc.gpsimd.memset(ms_cast(xv[64:128, 64:65, :]), 0.0)
                nc.gpsimd.memset(ms_cast(xv[:, 65:66, :]), 0.0)

                xap = xs.ap()
                for t0, t1 in ((1, 17), (17, 33), (33, 49), (49, 64)):
                    dram = xap[:, 2 * t0 - 1:2 * t1 - 1, :].rearrange(
                        "ci (t j) x -> j ci t x", j=2
                    )
                    for j in (0, 1):
                        nc.sync.dma_start(
                            out=xv[j * 64:(j + 1) * 64, t0:t1, 1:129],
                            in_=dram[j],
                        )
                nc.sync.dma_start(out=xv[64:128, 0:1, 1:129], in_=xap[:, 0:1, :])
                nc.sync.dma_start(out=xv[0:64, 64:65, 1:129],
                                  in_=xap[:, 127:128, :])

                out_ap = out.ap().rearrange(
                    "co (s g i) x -> s i co g x", g=4, i=2)

                for half in range(2):
                    ptiles = [
                        psum_pool.tile(
                            [128, 4, 128], mybir.dt.float32, tag="ps",
                            name=f"ps_{_rep}_{half}_{k}",
                        )
                        for k in range(8)
                    ]
                    for bi, (d, v) in enumerate(blocks):
                        lhsT = wbuf[:, bi * 128:(bi + 1) * 128]
                        first = bi == 0
                        last = bi == nblk - 1
                        for sg in range(half * 8, half * 8 + 8):
                            t0 = 4 * sg + d
                            rhs = xv[:, t0:t0 + 4, v:v + 128]
                            nc.tensor.matmul(
                                ptiles[sg % 8][:], lhsT, rhs,
                                start=first, stop=last,
                            )
                    for sg in range(half * 8, half * 8 + 8):
                        st = stage_pool.tile(
                            [128, 4, 128], mybir.dt.float32, tag="st",
                            name=f"st_{_rep}_{half}_{sg}",
                        )
                        nc.vector.tensor_copy(st[:], ptiles[sg % 8][:])
                        for i in (0, 1):
                            nc.sync.dma_start(
                                out=out_ap[sg][i],
                                in_=st[i * 64:(i + 1) * 64],
                            )

    nc.compile()
    return nc


_CACHE = {}


def _get_nc(plan, repeat=1):
    key = (plan["key"], repeat)
    if key not in _CACHE:
        builder = build_fast if plan["mode"] == "fast" else build_general
        _CACHE[key] = builder(plan, repeat)
    return _CACHE[key]


# --------------------------------------------------------------------
# host-side staging
# --------------------------------------------------------------------

def make_in_maps(x, plan):
    """Per-core input dicts. Fast path: pack each image into the two
    row-layout SBUF images (bf16, zeros baked in)."""
    x = np.asarray(x)
    if plan["mode"] == "general":
        Whc = np.ascontiguousarray(plan["Wh"].astype(np.float32))
        return [
            {"xs": np.ascontiguousarray(x[b].astype(np.float32)), "wb": Whc}
            for b in range(B)
        ]
    Whc = np.ascontiguousarray(plan["Wh"].astype(BF16))
    xb16 = x.astype(BF16)
    in_maps = []
    for b in range(B):
        xp = np.zeros((CIN, 132, 132), dtype=BF16)
        xp[:, 1:129, 1:129] = xb16[b]
        A = np.ascontiguousarray(
            xp[:, 0:130, :].reshape(CIN, NSEG, 2, 132)
            .transpose(2, 0, 1, 3).reshape(128, NSEG * WP))
        Bm = np.ascontiguousarray(
            xp[:, 1:131, :].reshape(CIN, NSEG, 2, 132)
            .transpose(2, 0, 1, 3).reshape(128, NSEG * WP))
        in_maps.append({"xa": A, "xb2": Bm, "wb": Whc})
    return in_maps


def unpack_out(res, plan):
    """Per-core output dict -> [COUT, H, W] float32."""
    arr = np.asarray(res["out"])
    if plan["mode"] == "general":
        return arr.astype(np.float32)
    return np.ascontiguousarray(
        arr.reshape(2, 64, 16, 4, 128).transpose(1, 2, 3, 0, 4)
        .reshape(COUT, H, W)).astype(np.float32)


def kernel(x, weights, tap_offsets):
    x = np.asarray(x)
    assert x.shape == (B, CIN, H, W)
    plan = plan_from_inputs(weights, tap_offsets)
    nc = _get_nc(plan)
    in_maps = make_in_maps(x, plan)
    res = run_bass_kernel_spmd(nc, in_maps, list(range(NCORES)), trace=False)
    outs = [unpack_out(res.results[c], plan) for c in range(NCORES)]
    return np.stack(outs, 0).astype(np.float32)

